# revision 1
# baseline (speedup 1.0000x reference)
"""CrossNetMix (DCN-V2 MoE cross-network) Trainium2 Bass kernel.

Math (per layer i, reference):
    v    = tanh(x_l @ V[i])      per expert      (B, E, R)
    c    = tanh(v @ C[i].T)      per expert      (B, E, R)
    u    = c @ U[i].T            per expert      (B, E, D)
    gate = softmax(x_l @ G.T)                    (B, E)
    x_l  = sum_e gate_e * x0 * (u_e + bias[i]) + x_l

Since softmax gates sum to 1 the update is
    x_{l+1} = x0 * (sum_e gate_e u_e + bias[i]) + x_l
and with S_0 = 1, x_l = x0 * S_l where
    S_{l+1} = S_l + umix_l + bias[i],   umix = U_arr^T (gate256 * c)
so the residual/bias fold into cheap per-chunk elementwise ops.

Device layout: features on partitions, tokens on the free dim.  The host
pre-transposes each core's x slice to (D, Bc) so every DMA is contiguous
and the whole matmul chain (V -> C -> U) stays feature-major with zero
on-device transposes.  Gate softmax over the E=4 partition dim is done
with tiny matmuls (ones(4,4) broadcast-sum, selector broadcast 4->256).

All matmul operand tiles are float32r-typed (full PE rate; the BIR
verifier requires fp32r producers for fp32r matmuls).  The S recurrence
runs through the PSUM accumulator: the U matmul leaves umix in PSUM, an
identity matmul accumulates S_l on top, and one DVE scalar_tensor_tensor
materializes x_{l+1} = (psum + bias) * x0 straight from PSUM.  ACT
copies S_{l+1} = psum + bias to SBUF (fused per-partition bias) only
when the next layer's identity matmul needs it.
"""

import numpy as np

import concourse.bacc as bacc
import concourse.bass as bass
import concourse.mybir as mybir
import concourse.tile as tile
from concourse.bass_utils import run_bass_kernel_spmd

# Problem constants (hardcoded per contract).
B, D, LAYERS, E, R = 16384, 1024, 3, 4, 64
ER = E * R                  # 256
NCORES = 8
BC = B // NCORES            # 2048 tokens per core
NB = 512                    # token block = PSUM bank width (fp32)
KC = D // 128               # 8 feature chunks
F32 = mybir.dt.float32
F32R = mybir.dt.float32r

AF = mybir.ActivationFunctionType
OP = mybir.AluOpType


def _emit(tc, outT, xT, w1, gt, cw, ua, biasP, sel, onesE, eye, n_blocks):
    nc = tc.nc
    from contextlib import ExitStack

    with ExitStack() as ctx:
        consts = ctx.enter_context(tc.tile_pool(name="consts", bufs=1))
        xin = ctx.enter_context(tc.tile_pool(name="xin", bufs=2))
        work = ctx.enter_context(tc.tile_pool(name="work", bufs=2))
        pp = ctx.enter_context(tc.tile_pool(name="pp", bufs=2, space="PSUM"))

        # ---- resident weights ----
        w1_sb = []          # [L][KC] tiles (128, ER): lhsT for V matmul
        ua_sb = []          # [L][2] tiles (128, D): lhsT for U matmul
        cw_sb = []          # [L][2] tiles (128, 128): block-diag C^T
        bias_sb = []        # [L] tiles (128, KC): bias column per d-chunk
        for i in range(LAYERS):
            per_k = []
            for k in range(KC):
                t = consts.tile([128, ER], F32R, name=f"w1_{i}_{k}")
                nc.sync.dma_start(out=t, in_=w1[i, k * 128:(k + 1) * 128, :])
                per_k.append(t)
            w1_sb.append(per_k)

            per_kc = []
            for kc in range(2):
                t = consts.tile([128, D], F32R, name=f"ua_{i}_{kc}")
                nc.sync.dma_start(out=t, in_=ua[i, kc * 128:(kc + 1) * 128, :])
                per_kc.append(t)
            ua_sb.append(per_kc)

            per_j = []
            for j in range(2):
                t = consts.tile([128, 128], F32R, name=f"cw_{i}_{j}")
                nc.sync.dma_start(out=t, in_=cw[i, j])
                per_j.append(t)
            cw_sb.append(per_j)

            t = consts.tile([128, KC], F32, name=f"bias_{i}")
            nc.sync.dma_start(out=t, in_=biasP[i].rearrange("(m p) -> p m", p=128))
            bias_sb.append(t)

        gt_sb = []
        for k in range(KC):
            t = consts.tile([128, E], F32R, name=f"gt_{k}")
            nc.sync.dma_start(out=t, in_=gt[k * 128:(k + 1) * 128, :])
            gt_sb.append(t)

        sel_sb = consts.tile([E, ER], F32R, name="sel")
        nc.sync.dma_start(out=sel_sb, in_=sel)
        onesE_sb = consts.tile([E, E], F32R, name="onesE")
        nc.sync.dma_start(out=onesE_sb, in_=onesE)
        eye_sb = consts.tile([128, 128], F32R, name="eye")
        nc.sync.dma_start(out=eye_sb, in_=eye)

        xT_r = xT.rearrange("(k p) t -> p k t", p=128)
        outT_r = outT.rearrange("(m p) t -> p m t", p=128)

        # ---- token-block loop ----
        for b in range(n_blocks):
            x0 = xin.tile([128, KC, NB], F32R, tag="x0", name=f"x0_{b}")
            for k in range(KC):
                nc.sync.dma_start(out=x0[:, k, :],
                                  in_=xT_r[:, k, b * NB:(b + 1) * NB])

            S_prev = None
            xl = x0  # layer 0 input is x0 itself (S_0 = 1)
            for l in range(LAYERS):
                # gate logits: (E, NB) psum, accumulate over feature chunks
                glog = pp.tile([E, NB], F32, tag="small", bufs=1, name=f"glog{b}_{l}")
                for k in range(KC):
                    nc.tensor.matmul(glog, (gt_sb[k]), (xl[:, k, :]),
                                     start=(k == 0), stop=(k == KC - 1))

                # v = tanh(W1^T x): two 128-row er-chunks
                vps = [pp.tile([128, NB], F32, tag="vps", name=f"vps{b}_{l}_{j}") for j in range(2)]
                for j in range(2):
                    for k in range(KC):
                        nc.tensor.matmul(
                            vps[j],
                            (w1_sb[l][k][:, j * 128:(j + 1) * 128]),
                            (xl[:, k, :]),
                            start=(k == 0), stop=(k == KC - 1))
                v_sb = [work.tile([128, NB], F32R, tag=f"vsb{j}", name=f"vsb{b}_{l}_{j}") for j in range(2)]
                for j in range(2):
                    nc.scalar.activation(v_sb[j], vps[j], AF.Tanh)

                # softmax over E=4 partitions via matmul broadcast-sum
                expg = work.tile([E, NB], F32R, tag="expg", name=f"expg{b}_{l}")
                nc.scalar.activation(expg, glog, AF.Exp)
                sumb = pp.tile([E, NB], F32, tag="small", bufs=1, name=f"sumb{b}_{l}")
                nc.tensor.matmul(sumb, (onesE_sb), (expg), start=True, stop=True)
                recip = work.tile([E, NB], F32, tag="recip", name=f"recip{b}_{l}")
                nc.vector.reciprocal(recip, sumb)
                gate = work.tile([E, NB], F32R, tag="gate", name=f"gate{b}_{l}")
                nc.vector.tensor_mul(gate, expg.bitcast(F32), recip)

                # c = tanh(blockdiag(C^T) v)
                cps = [pp.tile([128, NB], F32, tag="mid", name=f"cps{b}_{l}_{j}") for j in range(2)]
                for j in range(2):
                    nc.tensor.matmul(cps[j], (cw_sb[l][j]), (v_sb[j]),
                                     start=True, stop=True)
                c_sb = [work.tile([128, NB], F32R, tag=f"csb{j}", name=f"csb{b}_{l}_{j}") for j in range(2)]
                for j in range(2):
                    nc.scalar.activation(c_sb[j], cps[j], AF.Tanh)

                # broadcast gate (E, NB) -> (ER, NB) with the selector matmul
                gps = [pp.tile([128, NB], F32, tag="mid", name=f"gps{b}_{l}_{j}") for j in range(2)]
                for j in range(2):
                    nc.tensor.matmul(gps[j], (sel_sb[:, j * 128:(j + 1) * 128]),
                                     (gate), start=True, stop=True)
                cg = [work.tile([128, NB], F32R, tag=f"cg{j}", name=f"cg{b}_{l}_{j}") for j in range(2)]
                for j in range(2):
                    nc.vector.tensor_mul(cg[j], c_sb[j].bitcast(F32), gps[j])

                # umix per d-chunk + S update.
                # S_{l+1} = umix + bias_l + S_l: S_l joins via an identity
                # matmul accumulating into the psum (PE), bias via the ACT
                # fused per-partition bias on the psum->SBUF copy, so DVE
                # only does the x0*S materialize.
                # The psum ends as umix + S_l (eye matmul); DVE then
                # materializes x_{l+1} = (psum + bias) * x0 in ONE op
                # (scalar_tensor_tensor), skipping the ACT hop on the
                # critical path.  ACT still copies S_new = psum + bias to
                # SBUF, but only when layer l+1 needs it for its eye
                # matmul (l < LAYERS-1) -- off the critical path.
                last = l == LAYERS - 1
                if not last:
                    S_new = work.tile([128, KC, NB], F32R, tag="S", bufs=2,
                                      name=f"S{b}_{l}")
                if last:
                    tgt = work.tile([128, KC, NB], F32, tag="xl", name=f"osb{b}")
                else:
                    tgt = work.tile([128, KC, NB], F32R, tag="xl", name=f"xl{b}_{l}")
                for m in range(KC):
                    ups = pp.tile([128, NB], F32, tag="ups", bufs=3, name=f"ups{b}_{l}_{m}")
                    for kc in range(2):
                        nc.tensor.matmul(
                            ups,
                            (ua_sb[l][kc][:, m * 128:(m + 1) * 128]),
                            (cg[kc]),
                            start=(kc == 0), stop=(kc == 1 and l == 0))
                    if l > 0:
                        nc.tensor.matmul(ups, eye_sb, S_prev[:, m, :],
                                         start=False, stop=True)
                    bcol = bias_sb[l][:, m:m + 1]
                    nc.vector.scalar_tensor_tensor(
                        out=tgt[:, m, :], in0=ups, scalar=bcol,
                        in1=x0[:, m, :].bitcast(F32),
                        op0=OP.add, op1=OP.mult)
                    if not last:
                        nc.scalar.activation(S_new[:, m, :], ups, AF.Identity,
                                             bias=bcol)
                    else:
                        nc.sync.dma_start(
                            out=outT_r[:, m, b * NB:(b + 1) * NB],
                            in_=tgt[:, m, :])
                if not last:
                    S_prev = S_new
                    xl = tgt


def build_bass(n_blocks=BC // NB):
    nc = bacc.Bacc(trn_type="TRN2", target_bir_lowering=False, debug=False)
    bc = n_blocks * NB

    def inp(name, shape, dt=F32R):
        return nc.dram_tensor(name, list(shape), dt, kind="ExternalInput").ap()

    xT = inp("xT", (D, bc))
    w1 = inp("w1", (LAYERS, D, ER))
    gt = inp("gt", (D, E))
    cw = inp("cw", (LAYERS, 2, 128, 128))
    ua = inp("ua", (LAYERS, ER, D))
    biasP = inp("biasP", (LAYERS, D), F32)
    sel = inp("sel", (E, ER))
    onesE = inp("onesE", (E, E))
    eye = inp("eye", (128, 128))
    outT = nc.dram_tensor("outT", [D, bc], F32, kind="ExternalOutput").ap()

    with tile.TileContext(nc) as tc:
        _emit(tc, outT, xT, w1, gt, cw, ua, biasP, sel, onesE, eye, n_blocks)
    nc.compile()
    return nc


def prep_weights(U, V, C, bias, G):
    """Host-side weight rearrangement (replicated across cores)."""
    U = np.asarray(U, np.float32)
    V = np.asarray(V, np.float32)
    C = np.asarray(C, np.float32)
    bias = np.asarray(bias, np.float32)
    G = np.asarray(G, np.float32)

    # w1[i, d, e*R+r] = V[i, e, d, r]
    w1 = np.ascontiguousarray(V.transpose(0, 2, 1, 3).reshape(LAYERS, D, ER))
    # ua[i, e*R+r, d] = U[i, e, d, r]
    ua = np.ascontiguousarray(U.transpose(0, 1, 3, 2).reshape(LAYERS, ER, D))
    # block-diagonal C^T chunks: chunk j holds experts 2j, 2j+1
    cw = np.zeros((LAYERS, 2, 128, 128), np.float32)
    for i in range(LAYERS):
        for e in range(E):
            j, o = divmod(e, 2)
            cw[i, j, o * R:(o + 1) * R, o * R:(o + 1) * R] = C[i, e].T
    gt = np.ascontiguousarray(G.T)
    biasP = bias.copy()
    biasP[0] += 1.0  # S_0 = 1 folded into layer-0 bias
    sel = np.zeros((E, ER), np.float32)
    for e in range(E):
        sel[e, e * R:(e + 1) * R] = 1.0
    onesE = np.ones((E, E), np.float32)
    eye = np.eye(128, dtype=np.float32)
    return dict(w1=w1, gt=gt, cw=cw, ua=ua, biasP=biasP, sel=sel,
                onesE=onesE, eye=eye)


_NC_CACHE = {}


def _get_nc(n_blocks):
    if n_blocks not in _NC_CACHE:
        _NC_CACHE[n_blocks] = build_bass(n_blocks)
    return _NC_CACHE[n_blocks]


def run(inputs, trace=False, **spmd_kwargs):
    """Shard, run on 8 cores, gather.  Returns (output, BassKernelResults)."""
    x = np.asarray(inputs["x"], np.float32)
    weights = prep_weights(inputs["U"], inputs["V"], inputs["C"],
                           inputs["bias"], inputs["G"])
    nc = _get_nc(BC // NB)

    in_maps = []
    for c in range(NCORES):
        xc = np.ascontiguousarray(x[c * BC:(c + 1) * BC].T)  # (D, BC)
        in_maps.append(dict(xT=xc, **weights))

    res = run_bass_kernel_spmd(nc, in_maps, core_ids=list(range(NCORES)),
                               trace=trace, **spmd_kwargs)

    out = np.empty((B, D), np.float32)
    for c in range(NCORES):
        out[c * BC:(c + 1) * BC] = res.results[c]["outT"].T
    return out, res


def kernel(**inputs):
    out, _ = run(inputs)
    return out



# revision 29
# speedup vs baseline: 1.3403x; 1.3403x over previous
"""CrossNetMix (DCN-V2 MoE cross-network) Trainium2 Bass kernel, v2.

Math (per layer i):
    v    = tanh(x_l @ V[i])      per expert      (B, E, R)
    c    = tanh(v @ C[i].T)      per expert      (B, E, R)
    u    = c @ U[i].T            per expert      (B, E, D)
    gate = softmax(x_l @ G.T)                    (B, E)
    x_l  = sum_e gate_e * x0 * (u_e + bias[i]) + x_l

Using sum_e gate_e = 1 and S_0 = 1:
    S_{l+1} = S_l + umix_l + bias[i],  umix = U^T (gate_bcast * c)
    x_{l+1} = x0 * S_{l+1}

v2 design (all-bf16 matmuls; engine-balanced elementwise):
  * Every matmul operand is bf16 (same PE cost as fp32r in the TRN2 cost
    model, but halves DMA/SBUF).  fp8 was measured at rel-err 1.5-3.3e-2
    vs the 2e-2 gate - too risky; bf16 sits at ~2-6e-3.
  * Gate logits via 32 tiny token-major matmuls (lhsT = x_l 128-token
    slice, rhs = G chunk (128,4)) into a (128,16) psum: 2ns each vs
    8x213ns for the feature-major version.  Softmax runs token-major
    ((128,16) tiles: one exp, one grouped reduce, reciprocal, 4
    tensor_scalar normalizes), then 4 PE transposes restore (4, NB).
  * No identity matmul for the S recurrence: S_new = (ups + bias) + S_prev
    runs as per-chunk scalar_tensor_tensor split across Pool (gpsimd) and
    DVE; x_{l+1} = S_new * x0 is a pure-bf16 SBUF tensor_tensor (DVE 2x
    mode) done per chunk-pair.
  * Layer-major emission with a one-block software pipeline lag so PE
    never waits on ACT tanh: for each layer, front-half(b) = gate tinies +
    V matmuls, back-half(b-1) = transposes + C + sel + U.

Engine budget per layer-block (ns): PE 7944, DVE ~5500, Pool ~4800,
ACT ~2800-5000, for a PE-bound total of ~95-110us/core (vs 196us v1).
"""

import numpy as np
import ml_dtypes

import concourse.bacc as bacc
import concourse.bass as bass
import concourse.mybir as mybir
import concourse.tile as tile
from concourse.bass_utils import run_bass_kernel_spmd

# Problem constants (hardcoded per contract).
B, D, LAYERS, E, R = 16384, 1024, 3, 4, 64
ER = E * R                  # 256
NCORES = 8
BC = B // NCORES            # 2048 tokens per core
NB = 512                    # token block = PSUM bank width (fp32)
KC = D // 128               # 8 feature chunks
NTC = NB // 128             # 4 token chunks per block (for gate)
F32 = mybir.dt.float32
BF16 = mybir.dt.bfloat16

AF = mybir.ActivationFunctionType
OP = mybir.AluOpType


def _emit(tc, outT, xT, w1, gt, cw, ua, biasP, sel, eye, n_blocks):
    nc = tc.nc
    from contextlib import ExitStack

    with ExitStack() as ctx:
        consts = ctx.enter_context(tc.tile_pool(name="consts", bufs=1))
        xin = ctx.enter_context(tc.tile_pool(name="xin", bufs=1))
        xlp = ctx.enter_context(tc.tile_pool(name="xlp", bufs=2))
        sp = ctx.enter_context(tc.tile_pool(name="sp", bufs=2))
        work = ctx.enter_context(tc.tile_pool(name="work", bufs=2))
        # psum pools -- 8 banks total, budgeted exactly:
        # glogT/gate_ps shared ring 1 + vps 2x2 + cps/gps shared ring 1
        # + ups 2 = 8
        pp_small = ctx.enter_context(tc.tile_pool(name="pps", bufs=1, space="PSUM"))
        pp_v = ctx.enter_context(tc.tile_pool(name="ppv", bufs=2, space="PSUM"))
        pp_cg = ctx.enter_context(tc.tile_pool(name="ppcg", bufs=1, space="PSUM"))
        pp_u = ctx.enter_context(tc.tile_pool(name="ppu", bufs=2, space="PSUM"))

        # ---- resident weights (all bf16 except bias), loaded lazily:
        # gate/eye/sel + layer-0 weights + x0(b0) gate the first compute;
        # layer 1/2 weights stream in behind layer-0 compute.
        gt_sb = consts.tile([128, KC, E], BF16, name="gt")
        nc.sync.dma_start(out=gt_sb, in_=gt.rearrange("(k p) e -> p k e", p=128))
        eye_sb = consts.tile([128, 128], BF16, name="eye")
        nc.sync.dma_start(out=eye_sb, in_=eye)
        sel_sb = consts.tile([E, ER], BF16, name="sel")
        nc.sync.dma_start(out=sel_sb, in_=sel)

        w1_sb = {}          # [L][KC] (128, ER): lhsT for V matmul
        ua_sb = {}          # [L][2]  (128, D):  lhsT for U matmul
        cw_sb = {}          # [L][2]  (128, 128): block-diag C^T
        bias_sb = {}        # [L]     (128, KC) f32: bias column per d-chunk

        def load_layer_weights(i):
            # one DMA per weight group -- per-DMA overhead (~2.2us fixed)
            # dwarfs the data time for small transfers
            w1_sb[i] = consts.tile([128, KC, ER], BF16, name=f"w1_{i}")
            nc.sync.dma_start(out=w1_sb[i],
                              in_=w1[i].rearrange("(k p) e -> p k e", p=128))
            cw_sb[i] = consts.tile([128, 2, 128], BF16, name=f"cw_{i}")
            nc.sync.dma_start(out=cw_sb[i],
                              in_=cw[i].rearrange("j p c -> p j c"))
            ua_sb[i] = consts.tile([128, 2, D], BF16, name=f"ua_{i}")
            nc.sync.dma_start(out=ua_sb[i],
                              in_=ua[i].rearrange("(c p) d -> p c d", p=128))
            t = consts.tile([128, KC], F32, name=f"bias_{i}")
            nc.sync.dma_start(out=t, in_=biasP[i].rearrange("(m p) -> p m", p=128))
            bias_sb[i] = t

        xT_r = xT.rearrange("(k p) t -> p k t", p=128)
        outT_r = outT.rearrange("(m p) t -> p m t", p=128)

        x0s = [None] * n_blocks

        def load_x0(b):
            x0 = xin.tile([128, KC, NB], BF16, name=f"x0_{b}")
            nc.sync.dma_start(out=x0, in_=xT_r[:, :, b * NB:(b + 1) * NB])
            x0s[b] = x0

        xls = [None] * n_blocks  # layer input per block (x0 for layer 0)
        Ss = [None] * n_blocks   # S tile per block (None for layer 0)

        # Per-block state carried from front-half to back-half.
        pend = [None] * n_blocks

        def front(b, l):
            """Gate logits (token-major) + softmax minis + V matmuls."""
            xl = x0s[b] if l == 0 else xls[b]
            glogT = pp_small.tile([128, E * NTC], F32, tag="small",
                                  name=f"glT{b}_{l}")
            for tcn in range(NTC):
                sl = glogT[:, tcn * E:(tcn + 1) * E]
                for k in range(KC):
                    nc.tensor.matmul(sl,
                                     xl[:, k, tcn * 128:(tcn + 1) * 128],
                                     gt_sb[:, k, :],
                                     start=(k == 0), stop=(k == KC - 1))
            vps = pp_v.tile([128, 2, NB], F32, tag="vps", name=f"vps{b}_{l}")
            for j in range(2):
                for k in range(KC):
                    nc.tensor.matmul(
                        vps[:, j, :],
                        w1_sb[l][:, k, j * 128:(j + 1) * 128],
                        xl[:, k, :],
                        start=(k == 0), stop=(k == KC - 1))

            # softmax (token-major): exp -> grouped sum -> recip -> scale
            expT = work.tile([128, NTC, E], BF16, tag="expT", name=f"expT{b}_{l}")
            nc.scalar.activation(expT.rearrange("p k e -> p (k e)"), glogT, AF.Exp)
            sumT = work.tile([128, NTC], F32, tag="sumT", name=f"sumT{b}_{l}")
            nc.vector.tensor_reduce(sumT, expT, mybir.AxisListType.X, OP.add)
            recT = work.tile([128, NTC], F32, tag="recT", name=f"recT{b}_{l}")
            nc.vector.reciprocal(recT, sumT)
            gateT = work.tile([128, NTC, E], BF16, tag="gateT", name=f"gaT{b}_{l}")
            for tcn in range(NTC):
                nc.vector.tensor_scalar(
                    out=gateT[:, tcn, :], in0=expT[:, tcn, :],
                    scalar1=recT[:, tcn:tcn + 1], scalar2=None, op0=OP.mult)

            # transposes sit after the V matmuls in the PE stream: the DVE
            # minis are long done by then, so PE never waits; gate_sb's ACT
            # copy lands ahead of tanh-v so back()'s sel matmul isn't stuck
            # behind the next block's tanh in the in-order ACT queue.
            gate_ps = pp_small.tile([E, NB], BF16, tag="small",
                                    name=f"gps{b}_{l}")
            for tcn in range(NTC):
                nc.tensor.transpose(gate_ps[:, tcn * 128:(tcn + 1) * 128],
                                    gateT[:, tcn, :], eye_sb)
            gate_sb = work.tile([E, NB], BF16, tag="gate_sb", name=f"gsb{b}_{l}")
            nc.scalar.activation(gate_sb, gate_ps, AF.Identity)

            v_sb = work.tile([128, 2, NB], BF16, tag="vsb", bufs=3,
                             name=f"vsb{b}_{l}")
            nc.scalar.activation(v_sb.rearrange("p j t -> p (j t)"),
                                 vps.rearrange("p j t -> p (j t)"), AF.Tanh)
            pend[b] = (gate_sb, v_sb)

        def back(b, l):
            """C + sel + U + S update + x_{l+1}."""
            gate_sb, v_sb = pend[b]
            x0 = x0s[b]
            last = l == LAYERS - 1

            # c = tanh(blockdiag(C^T) v); cg = c * bcast(gate)
            cg = work.tile([128, 2, NB], BF16, tag="cg", name=f"cg{b}_{l}")
            for j in range(2):
                cps = pp_cg.tile([128, NB], F32, tag="cgps", name=f"cps{b}_{l}_{j}")
                nc.tensor.matmul(cps, cw_sb[l][:, j, :], v_sb[:, j, :],
                                 start=True, stop=True)
                c_sb = work.tile([128, NB], BF16, tag=f"csb{j}",
                                 name=f"csb{b}_{l}_{j}")
                nc.scalar.activation(c_sb, cps, AF.Tanh)
                # broadcast gate (E, NB) -> (128, NB) for this er-chunk
                # (sel chunk j selects experts 2j, 2j+1)
                gps = pp_cg.tile([128, NB], F32, tag="cgps", name=f"gg{b}_{l}_{j}")
                nc.tensor.matmul(gps, sel_sb[:, j * 128:(j + 1) * 128],
                                 gate_sb, start=True, stop=True)
                nc.vector.tensor_mul(cg[:, j, :], c_sb, gps)

            # U matmuls + S update + x_{l+1} = S_new * x0
            S_prev = Ss[b]
            S_new = sp.tile([128, KC, NB], BF16, tag=f"S{b}", name=f"S{b}_{l}")
            xl_new = xlp.tile([128, KC, NB], BF16, tag=f"xl{b}",
                              name=f"xl{b}_{l}")
            for m in range(KC):
                ups = pp_u.tile([128, NB], F32, tag="ups", name=f"ups{b}_{l}_{m}")
                for kc in range(2):
                    nc.tensor.matmul(
                        ups,
                        ua_sb[l][:, kc, m * 128:(m + 1) * 128],
                        cg[:, kc, :],
                        start=(kc == 0), stop=(kc == 1))
                bcol = bias_sb[l][:, m:m + 1]
                # GPSIMD cannot touch PSUM, so Pool only ever sees SBUF
                # operands: its accum chunks go through an ACT psum->SBUF
                # copy (which adds the bias for free) first.
                if l == 0:
                    # S_1 = ups + (bias_0 + 1): one op, DVE 3 / ACT 5.
                    if m in (0, 1, 2):
                        nc.vector.tensor_scalar(
                            out=S_new[:, m, :], in0=ups, scalar1=bcol,
                            scalar2=None, op0=OP.add)
                    else:
                        nc.scalar.activation(S_new[:, m, :], ups, AF.Identity,
                                             bias=bcol)
                else:
                    # S_{l+1} = (ups + bias_l) + S_l: DVE stt for 5 chunks,
                    # ACT-copy + Pool-add for 3.
                    if m in (0, 1, 2, 5, 6):
                        nc.vector.scalar_tensor_tensor(
                            out=S_new[:, m, :], in0=ups, scalar=bcol,
                            in1=S_prev[:, m, :], op0=OP.add, op1=OP.add)
                    else:
                        u_sb = work.tile([128, NB], BF16, tag="usb",
                                         name=f"usb{b}_{l}_{m}")
                        nc.scalar.activation(u_sb, ups, AF.Identity, bias=bcol)
                        nc.gpsimd.tensor_add(S_new[:, m, :], u_sb,
                                             S_prev[:, m, :])
                if m % 2 == 1:
                    # pure-bf16 SBUF pair multiply (DVE 2x mode; last pair
                    # on Pool to shave the DVE queue)
                    eng = nc.gpsimd if m == 7 else nc.vector
                    eng.tensor_mul(xl_new[:, m - 1:m + 1, :],
                                   S_new[:, m - 1:m + 1, :],
                                   x0[:, m - 1:m + 1, :])
                    if last and m % 4 == 3:
                        nc.sync.dma_start(
                            out=outT_r[:, m - 3:m + 1, b * NB:(b + 1) * NB],
                            in_=xl_new[:, m - 3:m + 1, :])
            Ss[b] = S_new
            xls[b] = xl_new
            pend[b] = None

        # ---- layer-major, two-block-lag pipeline ----
        load_layer_weights(0)
        load_x0(0)
        for l in range(LAYERS):
            done = 0
            for b in range(n_blocks):
                if l == 0 and b + 1 < n_blocks:
                    load_x0(b + 1)  # stream remaining inputs behind compute
                front(b, l)
                if l + 1 < LAYERS and b == 0:
                    load_layer_weights(l + 1)  # hide next layer's weights
                if b >= 2:
                    back(done, l)
                    done += 1
            while done < n_blocks:
                back(done, l)
                done += 1


def build_bass(n_blocks=BC // NB):
    nc = bacc.Bacc(trn_type="TRN2", target_bir_lowering=False, debug=False)
    bc = n_blocks * NB

    def inp(name, shape, dt=BF16):
        return nc.dram_tensor(name, list(shape), dt, kind="ExternalInput").ap()

    xT = inp("xT", (D, bc))
    w1 = inp("w1", (LAYERS, D, ER))
    gt = inp("gt", (D, E))
    cw = inp("cw", (LAYERS, 2, 128, 128))
    ua = inp("ua", (LAYERS, ER, D))
    biasP = inp("biasP", (LAYERS, D), F32)
    sel = inp("sel", (E, ER))
    eye = inp("eye", (128, 128))
    outT = nc.dram_tensor("outT", [D, bc], BF16, kind="ExternalOutput").ap()

    with tile.TileContext(nc) as tc:
        _emit(tc, outT, xT, w1, gt, cw, ua, biasP, sel, eye, n_blocks)
    nc.compile()
    return nc


def _bf16(a):
    return np.asarray(a, np.float32).astype(ml_dtypes.bfloat16)


def prep_weights(U, V, C, bias, G):
    """Host-side weight rearrangement (replicated across cores)."""
    U = np.asarray(U, np.float32)
    V = np.asarray(V, np.float32)
    C = np.asarray(C, np.float32)
    bias = np.asarray(bias, np.float32)
    G = np.asarray(G, np.float32)

    # w1[i, d, e*R+r] = V[i, e, d, r]
    w1 = np.ascontiguousarray(V.transpose(0, 2, 1, 3).reshape(LAYERS, D, ER))
    # ua[i, e*R+r, d] = U[i, e, d, r]
    ua = np.ascontiguousarray(U.transpose(0, 1, 3, 2).reshape(LAYERS, ER, D))
    # block-diagonal C^T chunks: chunk j holds experts 2j, 2j+1
    cw = np.zeros((LAYERS, 2, 128, 128), np.float32)
    for i in range(LAYERS):
        for e in range(E):
            j, o = divmod(e, 2)
            cw[i, j, o * R:(o + 1) * R, o * R:(o + 1) * R] = C[i, e].T
    gt = np.ascontiguousarray(G.T)
    biasP = bias.copy()
    biasP[0] += 1.0  # S_0 = 1 folded into layer-0 bias
    sel = np.zeros((E, ER), np.float32)
    for e in range(E):
        sel[e, e * R:(e + 1) * R] = 1.0
    eye = np.eye(128, dtype=np.float32)
    return dict(w1=_bf16(w1), gt=_bf16(gt), cw=_bf16(cw), ua=_bf16(ua),
                biasP=np.asarray(biasP, np.float32), sel=_bf16(sel),
                eye=_bf16(eye))


_NC_CACHE = {}


def _get_nc(n_blocks):
    if n_blocks not in _NC_CACHE:
        _NC_CACHE[n_blocks] = build_bass(n_blocks)
    return _NC_CACHE[n_blocks]


def run(inputs, trace=False, **spmd_kwargs):
    """Shard, run on 8 cores, gather.  Returns (output, BassKernelResults)."""
    x = np.asarray(inputs["x"], np.float32)
    weights = prep_weights(inputs["U"], inputs["V"], inputs["C"],
                           inputs["bias"], inputs["G"])
    nc = _get_nc(BC // NB)

    in_maps = []
    for c in range(NCORES):
        xc = np.ascontiguousarray(x[c * BC:(c + 1) * BC].T).astype(
            ml_dtypes.bfloat16)  # (D, BC) bf16
        in_maps.append(dict(xT=xc, **weights))

    res = run_bass_kernel_spmd(nc, in_maps, core_ids=list(range(NCORES)),
                               trace=trace, **spmd_kwargs)

    out = np.empty((B, D), np.float32)
    for c in range(NCORES):
        out[c * BC:(c + 1) * BC] = res.results[c]["outT"].astype(np.float32).T
    return out, res


def kernel(**inputs):
    out, _ = run(inputs)
    return out
